# revision 18
# baseline (speedup 1.0000x reference)
"""Multi-head attention (nn_Attention_18528488915211) on 8 Trainium2 NeuronCores.

Sharding: tensor-parallel over heads. 16 heads / 8 cores = 2 heads per core.
Each core computes Q/K/V projections for its 256 columns of Wq/Wk/Wv,
attention for its 2 heads, and a partial output projection with its 256 rows
of Wo. The host sums the 8 partial outputs (the TP all-reduce) and adds bo.

v6 design (fp16/bf16 datapath, PE-bound software-pipelined schedule):
  - All 16-bit data; every matmul is [128,128] stationary x [128,512] moving
    at 1 cycle/row, so LDWEIGHTS (107ns) hides behind each 213ns matmul.
  - Q^T/K^T/V^T projections weights-stationary; V natural layout for AV is
    produced by the DMA XBAR transpose (SBUF->SBUF), costing no engine time.
  - xt and out live in DRAM as [*, 128, 512] tile-contiguous blocks (the
    host reshapes once) so every tile DMA is one 128KB contiguous run
    instead of 128 separate 1KB strided descriptors.
  - Attention per 512-query chunk (ic), heads interleaved, AV pipelined one
    key-block behind the Scalar-engine exp; output-projection matmuls of the
    previous chunk fill PE slack from slot 4 on (4/slot first, then 2/slot,
    so the slot-0-3 Scalar-engine deficit is recovered quickly).  Each
    chunk's tail (last AV, rowsum, reciprocal, normalize) is deferred past
    the next chunk's first S-pair so the Scalar engine never drains; the
    last chunk of a batch defers its tail into the next batch's first
    projection group.
  - xt tiles for batch b+1 prefetch in 4-tile groups at slots 0/4/8/12 of
    B(b) chunks, so output DMAs never queue behind a multi-MB burst.
  - PSUM banks: st ring 2 + ot_h0/h1 2x2 + shared proj/out/rowsum ring 2 = 8.
"""

import numpy as np

P = 128          # partitions
DM = 2048        # dmodel
DH = 128         # dhead
HPC = 2          # heads per core
DC = HPC * DH    # dmodel columns per core (256)
B = 4            # batch
L = 2048         # sequence length
T = B * L        # total tokens (8192)
KS = DM // P     # contraction subtiles (16)
TC = 512         # token/query chunk (matmul moving dim)
NCH = L // TC    # chunks per batch (4)
NJ = L // P      # key blocks per batch (16)
NCORES = 8


def _build_nc():
    import concourse.mybir as mybir
    import concourse.tile as tile
    from concourse import bacc

    f32 = mybir.dt.float32
    f16 = mybir.dt.float16
    bf16 = mybir.dt.bfloat16
    EXP = mybir.ActivationFunctionType.Exp

    nc = bacc.Bacc("TRN2", target_bir_lowering=False, debug=False,
                   num_devices=NCORES)

    # xt: [KS, B*NCH, P, TC] tile-contiguous; out: [T//P, DM//TC, P, TC]
    xt = nc.dram_tensor("xt", [KS, B * NCH, P, TC], f16,
                        kind="ExternalInput").ap()
    wq = nc.dram_tensor("wq", [DM, DC], f16, kind="ExternalInput").ap()
    wk = nc.dram_tensor("wk", [DM, DC], f16, kind="ExternalInput").ap()
    wv = nc.dram_tensor("wv", [DM, DC], f16, kind="ExternalInput").ap()
    bq = nc.dram_tensor("bq", [DC], f32, kind="ExternalInput").ap()
    bk = nc.dram_tensor("bk", [DC], f32, kind="ExternalInput").ap()
    bv = nc.dram_tensor("bv", [DC], f32, kind="ExternalInput").ap()
    wo = nc.dram_tensor("wo", [DC, DM], f16, kind="ExternalInput").ap()
    out = nc.dram_tensor("out", [T // P, DM // TC, P, TC], f16,
                         kind="ExternalOutput").ap()

    with tile.TileContext(nc) as tc:
        with (
            tc.tile_pool(name="wpool", bufs=1) as wpool,
            tc.tile_pool(name="xpool", bufs=40) as xpool,
            tc.tile_pool(name="qkv", bufs=2) as qkv,
            tc.tile_pool(name="misc", bufs=2) as misc,
            tc.tile_pool(name="psum", bufs=2, space="PSUM") as psum,
        ):
            xt_cache = {}

            def _claim_part(b, c, ks_lo, ks_hi):
                lst = xt_cache.setdefault((b, c), [])
                for ks in range(ks_lo, ks_hi):
                    xt_t = xpool.tile([P, TC], f16, tag="xt")
                    nc.sync.dma_start(xt_t[:], xt[ks, b * NCH + c])
                    lst.append(xt_t)

            def load_chunk(b, c):
                lst = xt_cache.pop((b, c), [])
                if len(lst) < KS:
                    xt_cache[(b, c)] = lst
                    _claim_part(b, c, len(lst), KS)
                    lst = xt_cache.pop((b, c))
                return lst

            def prefetch_part(b, c, quarter):
                if b < B:
                    n = len(xt_cache.get((b, c), []))
                    want = (quarter + 1) * 4
                    if n < want:
                        _claim_part(b, c, n, want)

            # --- resident weights (ordered so the first projection group
            # can start as early as possible) ---
            wq_sb = wpool.tile([P, KS, DC], f16, tag="wq")
            wk_sb = wpool.tile([P, KS, DC], f16, tag="wk")
            wv_sb = wpool.tile([P, KS, DC], f16, tag="wv")
            bq_sb = wpool.tile([P, HPC], f32, tag="bq")
            bk_sb = wpool.tile([P, HPC], f32, tag="bk")
            bv_sb = wpool.tile([P, HPC], f32, tag="bv")
            for ks in range(KS):
                nc.sync.dma_start(wq_sb[:, ks, :], wq[ks * P:(ks + 1) * P, :])
            nc.sync.dma_start(bq_sb[:], bq.rearrange("(h d) -> d h", d=P))
            _claim_part(0, 0, 0, KS)
            for ks in range(KS):
                nc.sync.dma_start(wk_sb[:, ks, :], wk[ks * P:(ks + 1) * P, :])
            nc.sync.dma_start(bk_sb[:], bk.rearrange("(h d) -> d h", d=P))
            _claim_part(0, 1, 0, KS)
            for ks in range(KS):
                nc.sync.dma_start(wv_sb[:, ks, :], wv[ks * P:(ks + 1) * P, :])
            nc.sync.dma_start(bv_sb[:], bv.rearrange("(h d) -> d h", d=P))
            _claim_part(0, 2, 0, KS)
            _claim_part(0, 3, 0, KS)
            ones_sb = wpool.tile([P, P], bf16, tag="ones")
            nc.any.memset(ones_sb[:], 1.0)
            wo_sb = wpool.tile([P, HPC, DM], f16, tag="wo")
            nc.sync.dma_start(wo_sb[:], wo.rearrange("(h p) n -> p h n", p=P))

            # Output-projection work for one finished 512-token chunk,
            # emitted 1 matmul per yield (pumped as PE filler work).
            def o_work_gen(ot_sb, qoff, tbg0):
                for tb in range(TC // P):
                    tsl = slice(qoff + tb * P, qoff + (tb + 1) * P)
                    for ncl in range(DM // TC):
                        o_ps = psum.tile([P, TC], f32, tag="ps", name="o_ps")
                        nc.tensor.matmul(
                            o_ps[:], ot_sb[:, 0, tsl],
                            wo_sb[:, 0, ncl * TC:(ncl + 1) * TC],
                            start=True, stop=False,
                        )
                        yield
                        nc.tensor.matmul(
                            o_ps[:], ot_sb[:, 1, tsl],
                            wo_sb[:, 1, ncl * TC:(ncl + 1) * TC],
                            start=False, stop=True,
                        )
                        o_sb = misc.tile([P, TC], f16, tag="oout",
                                         name="o_sb", bufs=6)
                        nc.vector.tensor_copy(o_sb[:], o_ps[:])
                        nc.sync.dma_start(out[tbg0 + tb, ncl], o_sb[:])
                        yield

            o_gens = []
            wu = [0]  # work-unit counter: +1 per B slot / A group

            def pump(n):
                while n > 0 and o_gens:
                    try:
                        next(o_gens[0][1])
                        n -= 1
                    except StopIteration:
                        o_gens.pop(0)

            def pump_old(n):
                # pump only if the head generator's chunk finished long ago
                # (its normalize is certain to have drained)
                if o_gens and wu[0] - o_gens[0][0] >= 20:
                    pump(n)

            # Deferred per-chunk tail: last AV pair, rowsum, recip, normalize.
            pending_tail = [None]

            def run_tail():
                if pending_tail[0] is not None:
                    t, pending_tail[0] = pending_tail[0], None
                    t()

            def make_tail(vn, ot0, ot1, racc, pt_last, ot, qs, qoff, tbg0):
                def tail():
                    nc.tensor.matmul(ot0[:], vn[:, NJ - 1, 0, :],
                                     pt_last[:, 0, :], start=False, stop=True)
                    nc.tensor.matmul(ot1[:], vn[:, NJ - 1, 1, :],
                                     pt_last[:, 1, :], start=False, stop=True)
                    rs0 = psum.tile([P, TC], f32, tag="ps", name="rs")
                    nc.tensor.matmul(rs0[:], ones_sb[:], racc[:, 0, :],
                                     start=True, stop=True)
                    rs1 = psum.tile([P, TC], f32, tag="ps", name="rs")
                    nc.tensor.matmul(rs1[:], ones_sb[:], racc[:, 1, :],
                                     start=True, stop=True)
                    rcp0 = misc.tile([P, TC], f32, tag="rcp", name="rcp")
                    nc.vector.reciprocal_approx_fast(rcp0[:], rs0[:])
                    nc.vector.tensor_mul(ot[:, 0, qs], ot0[:], rcp0[:])
                    rcp1 = misc.tile([P, TC], f32, tag="rcp", name="rcp")
                    nc.vector.reciprocal_approx_fast(rcp1[:], rs1[:])
                    nc.vector.tensor_mul(ot[:, 1, qs], ot1[:], rcp1[:])
                    o_gens.append((wu[0], o_work_gen(ot, qoff, tbg0)))
                return tail

            for b in range(B):
                t0 = b * L
                qt = qkv.tile([P, HPC, L], f16, tag="qt", name="qt")
                kt = qkv.tile([P, HPC, L], f16, tag="kt", name="kt")
                vt = qkv.tile([P, HPC, L], bf16, tag="vt", name="vt")
                vn = qkv.tile([P, NJ, HPC, DH], bf16, tag="vn", name="vn")
                ot = qkv.tile([P, HPC, L], f16, tag="ot", name="ot")

                # ============ Phase A: Q^T/K^T/V^T projections ============
                for c in range(NCH):
                    cs = slice(c * TC, (c + 1) * TC)
                    xts = load_chunk(b, c)
                    for w_sb, b_sb, dest in ((wq_sb, bq_sb, qt),
                                             (wk_sb, bk_sb, kt),
                                             (wv_sb, bv_sb, vt)):
                        for h in range(HPC):
                            acc = psum.tile([P, TC], f32, tag="ps",
                                            name="proj")
                            for ks in range(KS):
                                nc.tensor.matmul(
                                    acc[:],
                                    w_sb[:, ks, h * DH:(h + 1) * DH],
                                    xts[ks][:],
                                    start=(ks == 0), stop=(ks == KS - 1),
                                )
                            nc.vector.tensor_scalar_add(
                                dest[:, h, cs], acc[:], b_sb[:, h:h + 1])
                            wu[0] += 1
                            # previous batch's last-chunk tail rides behind
                            # the first projection group of this batch
                            run_tail()
                    for h in range(HPC):
                        nc.sync.dma_start_transpose(
                            vn[:, c * (TC // P):(c + 1) * (TC // P), h, :],
                            vt[:, h, cs],
                        )

                # ===== Phase B: attention, with fused output projection =====
                for ic in range(NCH):
                    qs = slice(ic * TC, (ic + 1) * TC)
                    ot0 = psum.tile([P, TC], f32, tag="ot0", name="ot0")
                    ot1 = psum.tile([P, TC], f32, tag="ot1", name="ot1")
                    racc = misc.tile([P, HPC, TC], bf16, tag="racc",
                                     name="racc")
                    pt_prev = None
                    for js in range(NJ):
                        if js % 4 == 0:
                            prefetch_part(b + 1, ic, js // 4)
                        ksl = slice(js * P, (js + 1) * P)
                        st0 = psum.tile([P, TC], f32, tag="st", name="st")
                        nc.tensor.matmul(st0[:], kt[:, 0, ksl], qt[:, 0, qs],
                                         start=True, stop=True)
                        st1 = psum.tile([P, TC], f32, tag="st", name="st")
                        nc.tensor.matmul(st1[:], kt[:, 1, ksl], qt[:, 1, qs],
                                         start=True, stop=True)
                        if js == 1:
                            run_tail()  # previous chunk's tail
                        pt = misc.tile([P, HPC, TC], bf16, tag="pt",
                                       name="pt", bufs=3)
                        nc.scalar.activation(pt[:, 0, :], st0[:], EXP,
                                             scale=1.0 / DH)
                        nc.scalar.activation(pt[:, 1, :], st1[:], EXP,
                                             scale=1.0 / DH)
                        if js == 1:
                            nc.vector.tensor_add(racc[:, 0, :],
                                                 pt_prev[:, 0, :],
                                                 pt[:, 0, :])
                            nc.gpsimd.tensor_add(racc[:, 1, :],
                                                 pt_prev[:, 1, :],
                                                 pt[:, 1, :])
                        elif js > 1:
                            nc.vector.tensor_add(racc[:, 0, :], racc[:, 0, :],
                                                 pt[:, 0, :])
                            nc.gpsimd.tensor_add(racc[:, 1, :], racc[:, 1, :],
                                                 pt[:, 1, :])
                        if pt_prev is not None:
                            nc.tensor.matmul(
                                ot0[:], vn[:, js - 1, 0, :], pt_prev[:, 0, :],
                                start=(js == 1), stop=False)
                            nc.tensor.matmul(
                                ot1[:], vn[:, js - 1, 1, :], pt_prev[:, 1, :],
                                start=(js == 1), stop=False)
                        if 4 <= js < 8:
                            pump(4)
                        elif js >= 8:
                            pump(2)
                        pt_prev = pt
                    pending_tail[0] = make_tail(vn, ot0, ot1, racc, pt_prev,
                                                ot, qs, ic * TC,
                                                (t0 + ic * TC) // P)

            # final flush: last chunk's tail + remaining output projection
            run_tail()
            pump(1 << 30)

    nc.compile()
    return nc


_NC_CACHE = None


def kernel(**inputs: np.ndarray) -> np.ndarray:
    from concourse.bass_utils import run_bass_kernel_spmd

    global _NC_CACHE
    f16 = np.float16
    x = np.asarray(inputs["x"], dtype=np.float32)
    Wq, bq = np.asarray(inputs["Wq"]), np.asarray(inputs["bq"])
    Wk, bk = np.asarray(inputs["Wk"]), np.asarray(inputs["bk"])
    Wv, bv = np.asarray(inputs["Wv"]), np.asarray(inputs["bv"])
    Wo, bo = np.asarray(inputs["Wo"]), np.asarray(inputs["bo"])

    # xt tiled: [KS, B*NCH, P, TC]; xt[ks, ch, p, t] = x^T[ks*128+p, ch*512+t]
    xT = np.ascontiguousarray(x.reshape(T, DM).T).astype(f16)
    xt = np.ascontiguousarray(
        xT.reshape(KS, P, B * NCH, TC).transpose(0, 2, 1, 3))

    in_maps = []
    for c in range(NCORES):
        sl = slice(c * DC, (c + 1) * DC)
        in_maps.append({
            "xt": xt,
            "wq": np.ascontiguousarray(Wq[:, sl]).astype(f16),
            "wk": np.ascontiguousarray(Wk[:, sl]).astype(f16),
            "wv": np.ascontiguousarray(Wv[:, sl]).astype(f16),
            "bq": np.ascontiguousarray(bq[sl]).astype(np.float32),
            "bk": np.ascontiguousarray(bk[sl]).astype(np.float32),
            "bv": np.ascontiguousarray(bv[sl]).astype(np.float32),
            "wo": np.ascontiguousarray(Wo[sl, :]).astype(f16),
        })

    if _NC_CACHE is None:
        _NC_CACHE = _build_nc()
    res = run_bass_kernel_spmd(_NC_CACHE, in_maps, core_ids=list(range(NCORES)))

    # out tiled [T//P, DM//TC, P, TC] -> [T, DM]
    acc = res.results[0]["out"].astype(np.float32)
    for c in range(1, NCORES):
        acc = acc + res.results[c]["out"].astype(np.float32)
    acc = acc.transpose(0, 2, 1, 3).reshape(T, DM)
    acc = acc + bo[None, :].astype(np.float32)
    return acc.reshape(B, L, DM)


# revision 19
# speedup vs baseline: 1.0654x; 1.0654x over previous
"""Multi-head attention (nn_Attention_18528488915211) on 8 Trainium2 NeuronCores.

Sharding: tensor-parallel over heads. 16 heads / 8 cores = 2 heads per core.
Each core computes Q/K/V projections for its 256 columns of Wq/Wk/Wv,
attention for its 2 heads, and a partial output projection with its 256 rows
of Wo. The host sums the 8 partial outputs (the TP all-reduce) and adds bo.

v6 design (fp16/bf16 datapath, PE-bound software-pipelined schedule):
  - All 16-bit data; every matmul is [128,128] stationary x [128,512] moving
    at 1 cycle/row, so LDWEIGHTS (107ns) hides behind each 213ns matmul.
  - Q^T/K^T/V^T projections weights-stationary; V natural layout for AV is
    produced by the DMA XBAR transpose (SBUF->SBUF), costing no engine time.
  - xt and out live in DRAM as [*, 128, 512] tile-contiguous blocks (the
    host reshapes once) so every tile DMA is one 128KB contiguous run
    instead of 128 separate 1KB strided descriptors.
  - Attention per 512-query chunk (ic), heads interleaved, AV pipelined one
    key-block behind the Scalar-engine exp; output-projection matmuls of the
    previous chunk fill PE slack from slot 4 on (4/slot first, then 2/slot,
    so the slot-0-3 Scalar-engine deficit is recovered quickly).  Each
    chunk's tail (last AV, rowsum, reciprocal, normalize) is deferred past
    the next chunk's first S-pair so the Scalar engine never drains; the
    last chunk of a batch defers its tail into the next batch's first
    projection group.
  - xt tiles for batch b+1 prefetch in 4-tile groups at slots 0/4/8/12 of
    B(b) chunks, so output DMAs never queue behind a multi-MB burst.
  - PSUM banks: st ring 2 + ot_h0/h1 2x2 + shared proj/out/rowsum ring 2 = 8.
"""

import numpy as np

P = 128          # partitions
DM = 2048        # dmodel
DH = 128         # dhead
HPC = 2          # heads per core
DC = HPC * DH    # dmodel columns per core (256)
B = 4            # batch
L = 2048         # sequence length
T = B * L        # total tokens (8192)
KS = DM // P     # contraction subtiles (16)
KD = KS // 2     # double-row contraction subtiles (8)
TC = 512         # token/query chunk (matmul moving dim)
NCH = L // TC    # chunks per batch (4)
NJ = L // P      # key blocks per batch (16)
NCORES = 8
WSCALE = 32.0    # host pre-scale on Wq/Wk for fp8 dynamic range


def _build_nc():
    import concourse.mybir as mybir
    import concourse.tile as tile
    from concourse import bacc

    f32 = mybir.dt.float32
    f16 = mybir.dt.float16
    bf16 = mybir.dt.bfloat16
    f8 = mybir.dt.float8e4
    DR = mybir.MatmulPerfMode.DoubleRow
    EXP = mybir.ActivationFunctionType.Exp

    nc = bacc.Bacc("TRN2", target_bir_lowering=False, debug=False,
                   num_devices=NCORES)

    # xt: [KS, B*NCH, P, TC] tile-contiguous; out: [T//P, DM//TC, P, TC]
    xt = nc.dram_tensor("xt", [KS, B * NCH, P, TC], f16,
                        kind="ExternalInput").ap()
    xt8 = nc.dram_tensor("xt8", [KD, B * NCH, P, 2, TC], f8,
                         kind="ExternalInput").ap()
    wq8 = nc.dram_tensor("wq8", [DM, DC], f8, kind="ExternalInput").ap()
    wk8 = nc.dram_tensor("wk8", [DM, DC], f8, kind="ExternalInput").ap()
    wv = nc.dram_tensor("wv", [DM, DC], f16, kind="ExternalInput").ap()
    bq = nc.dram_tensor("bq", [DC], f32, kind="ExternalInput").ap()
    bk = nc.dram_tensor("bk", [DC], f32, kind="ExternalInput").ap()
    bv = nc.dram_tensor("bv", [DC], f32, kind="ExternalInput").ap()
    wo = nc.dram_tensor("wo", [DC, DM], f16, kind="ExternalInput").ap()
    out = nc.dram_tensor("out", [T // P, DM // TC, P, TC], f16,
                         kind="ExternalOutput").ap()

    with tile.TileContext(nc) as tc:
        with (
            tc.tile_pool(name="wpool", bufs=1) as wpool,
            tc.tile_pool(name="xpool", bufs=40) as xpool,
            tc.tile_pool(name="qkv", bufs=2) as qkv,
            tc.tile_pool(name="misc", bufs=2) as misc,
            tc.tile_pool(name="psum", bufs=2, space="PSUM") as psum,
        ):
            xt_cache = {}
            xt8_cache = {}

            def _claim_part(b, c, ks_lo, ks_hi):
                lst = xt_cache.setdefault((b, c), [])
                l8 = xt8_cache.setdefault((b, c), [])
                for kk in range(ks_lo // 2, ks_hi // 2):
                    x8_t = xpool.tile([P, 2, TC], f8, tag="xt8", bufs=24)
                    nc.sync.dma_start(x8_t[:], xt8[kk, b * NCH + c])
                    l8.append(x8_t)
                for ks in range(ks_lo, ks_hi):
                    xt_t = xpool.tile([P, TC], f16, tag="xt")
                    nc.sync.dma_start(xt_t[:], xt[ks, b * NCH + c])
                    lst.append(xt_t)

            def load_chunk(b, c):
                lst = xt_cache.get((b, c), [])
                if len(lst) < KS:
                    _claim_part(b, c, len(lst), KS)
                return xt8_cache.pop((b, c)), xt_cache.pop((b, c))

            def prefetch_part(b, c, quarter):
                if b < B:
                    n = len(xt_cache.get((b, c), []))
                    want = (quarter + 1) * 4
                    if n < want:
                        _claim_part(b, c, n, want)

            # --- resident weights (ordered so the first projection group
            # can start as early as possible) ---
            wq_sb = wpool.tile([P, KD, 2, DC], f8, tag="wq")
            wk_sb = wpool.tile([P, KD, 2, DC], f8, tag="wk")
            wv_sb = wpool.tile([P, KS, DC], f16, tag="wv")
            bq_sb = wpool.tile([P, HPC], f32, tag="bq")
            bk_sb = wpool.tile([P, HPC], f32, tag="bk")
            bv_sb = wpool.tile([P, HPC], f32, tag="bv")
            for kk in range(KD):
                for u in range(2):
                    nc.sync.dma_start(
                        wq_sb[:, kk, u, :],
                        wq8[(2 * kk + u) * P:(2 * kk + u + 1) * P, :])
            nc.sync.dma_start(bq_sb[:], bq.rearrange("(h d) -> d h", d=P))
            _claim_part(0, 0, 0, KS)
            for kk in range(KD):
                for u in range(2):
                    nc.sync.dma_start(
                        wk_sb[:, kk, u, :],
                        wk8[(2 * kk + u) * P:(2 * kk + u + 1) * P, :])
            nc.sync.dma_start(bk_sb[:], bk.rearrange("(h d) -> d h", d=P))
            _claim_part(0, 1, 0, KS)
            for ks in range(KS):
                nc.sync.dma_start(wv_sb[:, ks, :], wv[ks * P:(ks + 1) * P, :])
            nc.sync.dma_start(bv_sb[:], bv.rearrange("(h d) -> d h", d=P))
            _claim_part(0, 2, 0, KS)
            _claim_part(0, 3, 0, KS)
            ones_sb = wpool.tile([P, P], bf16, tag="ones")
            nc.any.memset(ones_sb[:], 1.0)
            wo_sb = wpool.tile([P, HPC, DM], f16, tag="wo")
            nc.sync.dma_start(wo_sb[:], wo.rearrange("(h p) n -> p h n", p=P))

            # Output-projection work for one finished 512-token chunk,
            # emitted 1 matmul per yield (pumped as PE filler work).
            def o_work_gen(ot_sb, qoff, tbg0):
                for tb in range(TC // P):
                    tsl = slice(qoff + tb * P, qoff + (tb + 1) * P)
                    for ncl in range(DM // TC):
                        o_ps = psum.tile([P, TC], f32, tag="ps", name="o_ps")
                        nc.tensor.matmul(
                            o_ps[:], ot_sb[:, 0, tsl],
                            wo_sb[:, 0, ncl * TC:(ncl + 1) * TC],
                            start=True, stop=False,
                        )
                        yield
                        nc.tensor.matmul(
                            o_ps[:], ot_sb[:, 1, tsl],
                            wo_sb[:, 1, ncl * TC:(ncl + 1) * TC],
                            start=False, stop=True,
                        )
                        o_sb = misc.tile([P, TC], f16, tag="oout",
                                         name="o_sb", bufs=6)
                        nc.vector.tensor_copy(o_sb[:], o_ps[:])
                        nc.sync.dma_start(out[tbg0 + tb, ncl], o_sb[:])
                        yield

            o_gens = []
            wu = [0]  # work-unit counter: +1 per B slot / A group

            def pump(n):
                while n > 0 and o_gens:
                    try:
                        next(o_gens[0][1])
                        n -= 1
                    except StopIteration:
                        o_gens.pop(0)

            def pump_old(n):
                # pump only if the head generator's chunk finished long ago
                # (its normalize is certain to have drained)
                if o_gens and wu[0] - o_gens[0][0] >= 20:
                    pump(n)

            # Deferred per-chunk tail: last AV pair, rowsum, recip, normalize.
            pending_tail = [None]

            def run_tail():
                if pending_tail[0] is not None:
                    t, pending_tail[0] = pending_tail[0], None
                    t()

            def make_tail(vn, ot0, ot1, racc, pt_last, ot, qs, qoff, tbg0):
                def tail():
                    nc.tensor.matmul(ot0[:], vn[:, NJ - 1, 0, :],
                                     pt_last[:, 0, :], start=False, stop=True)
                    nc.tensor.matmul(ot1[:], vn[:, NJ - 1, 1, :],
                                     pt_last[:, 1, :], start=False, stop=True)
                    rs0 = psum.tile([P, TC], f32, tag="ps", name="rs")
                    nc.tensor.matmul(rs0[:], ones_sb[:], racc[:, 0, :],
                                     start=True, stop=True)
                    rs1 = psum.tile([P, TC], f32, tag="ps", name="rs")
                    nc.tensor.matmul(rs1[:], ones_sb[:], racc[:, 1, :],
                                     start=True, stop=True)
                    rcp0 = misc.tile([P, TC], f32, tag="rcp", name="rcp")
                    nc.vector.reciprocal_approx_fast(rcp0[:], rs0[:])
                    nc.vector.tensor_mul(ot[:, 0, qs], ot0[:], rcp0[:])
                    rcp1 = misc.tile([P, TC], f32, tag="rcp", name="rcp")
                    nc.vector.reciprocal_approx_fast(rcp1[:], rs1[:])
                    nc.vector.tensor_mul(ot[:, 1, qs], ot1[:], rcp1[:])
                    o_gens.append((wu[0], o_work_gen(ot, qoff, tbg0)))
                return tail

            for b in range(B):
                t0 = b * L
                qt = qkv.tile([P, HPC, L], f16, tag="qt", name="qt")
                kt = qkv.tile([P, HPC, L], f16, tag="kt", name="kt")
                vt = qkv.tile([P, HPC, L], bf16, tag="vt", name="vt")
                vn = qkv.tile([P, NJ, HPC, DH], bf16, tag="vn", name="vn")
                ot = qkv.tile([P, HPC, L], f16, tag="ot", name="ot")

                # ============ Phase A: Q^T/K^T/V^T projections ============
                for c in range(NCH):
                    cs = slice(c * TC, (c + 1) * TC)
                    x8s, xts = load_chunk(b, c)
                    for w_sb, b_sb, dest in ((wq_sb, bq_sb, qt),
                                             (wk_sb, bk_sb, kt)):
                        for h in range(HPC):
                            acc = psum.tile([P, TC], f32, tag="ps",
                                            name="proj")
                            for kk in range(KD):
                                nc.tensor.matmul(
                                    acc[:],
                                    w_sb[:, kk, :, h * DH:(h + 1) * DH],
                                    x8s[kk][:],
                                    start=(kk == 0), stop=(kk == KD - 1),
                                    perf_mode=DR,
                                )
                            nc.vector.tensor_scalar_add(
                                dest[:, h, cs], acc[:], b_sb[:, h:h + 1])
                            wu[0] += 1
                            run_tail()
                    for h in range(HPC):
                        acc = psum.tile([P, TC], f32, tag="ps", name="proj")
                        for ks in range(KS):
                            nc.tensor.matmul(
                                acc[:],
                                wv_sb[:, ks, h * DH:(h + 1) * DH],
                                xts[ks][:],
                                start=(ks == 0), stop=(ks == KS - 1),
                            )
                        nc.vector.tensor_scalar_add(
                            vt[:, h, cs], acc[:], bv_sb[:, h:h + 1])
                        wu[0] += 1
                        run_tail()
                    for h in range(HPC):
                        nc.sync.dma_start_transpose(
                            vn[:, c * (TC // P):(c + 1) * (TC // P), h, :],
                            vt[:, h, cs],
                        )

                # ===== Phase B: attention, with fused output projection =====
                for ic in range(NCH):
                    qs = slice(ic * TC, (ic + 1) * TC)
                    ot0 = psum.tile([P, TC], f32, tag="ot0", name="ot0")
                    ot1 = psum.tile([P, TC], f32, tag="ot1", name="ot1")
                    racc = misc.tile([P, HPC, TC], bf16, tag="racc",
                                     name="racc")
                    pt_prev = None
                    for js in range(NJ):
                        if js % 4 == 0:
                            prefetch_part(b + 1, ic, js // 4)
                        ksl = slice(js * P, (js + 1) * P)
                        st0 = psum.tile([P, TC], f32, tag="st", name="st")
                        nc.tensor.matmul(st0[:], kt[:, 0, ksl], qt[:, 0, qs],
                                         start=True, stop=True)
                        st1 = psum.tile([P, TC], f32, tag="st", name="st")
                        nc.tensor.matmul(st1[:], kt[:, 1, ksl], qt[:, 1, qs],
                                         start=True, stop=True)
                        if js == 0:
                            run_tail()  # previous chunk's tail
                        pt = misc.tile([P, HPC, TC], bf16, tag="pt",
                                       name="pt", bufs=3)
                        nc.scalar.activation(pt[:, 0, :], st0[:], EXP,
                                             scale=1.0 / (DH * WSCALE ** 2))
                        nc.scalar.activation(pt[:, 1, :], st1[:], EXP,
                                             scale=1.0 / (DH * WSCALE ** 2))
                        if js == 1:
                            nc.vector.tensor_add(racc[:, 0, :],
                                                 pt_prev[:, 0, :],
                                                 pt[:, 0, :])
                            nc.gpsimd.tensor_add(racc[:, 1, :],
                                                 pt_prev[:, 1, :],
                                                 pt[:, 1, :])
                        elif js > 1:
                            nc.vector.tensor_add(racc[:, 0, :], racc[:, 0, :],
                                                 pt[:, 0, :])
                            nc.gpsimd.tensor_add(racc[:, 1, :], racc[:, 1, :],
                                                 pt[:, 1, :])
                        if pt_prev is not None:
                            nc.tensor.matmul(
                                ot0[:], vn[:, js - 1, 0, :], pt_prev[:, 0, :],
                                start=(js == 1), stop=False)
                            nc.tensor.matmul(
                                ot1[:], vn[:, js - 1, 1, :], pt_prev[:, 1, :],
                                start=(js == 1), stop=False)
                        if 4 <= js < 8:
                            pump(4)
                        elif js >= 8:
                            pump(2)
                        pt_prev = pt
                    pending_tail[0] = make_tail(vn, ot0, ot1, racc, pt_prev,
                                                ot, qs, ic * TC,
                                                (t0 + ic * TC) // P)

            # final flush: last chunk's tail + remaining output projection
            run_tail()
            pump(1 << 30)

    nc.compile()
    return nc


_NC_CACHE = None


def kernel(**inputs: np.ndarray) -> np.ndarray:
    from concourse.bass_utils import run_bass_kernel_spmd

    global _NC_CACHE
    f16 = np.float16
    x = np.asarray(inputs["x"], dtype=np.float32)
    Wq, bq = np.asarray(inputs["Wq"]), np.asarray(inputs["bq"])
    Wk, bk = np.asarray(inputs["Wk"]), np.asarray(inputs["bk"])
    Wv, bv = np.asarray(inputs["Wv"]), np.asarray(inputs["bv"])
    Wo, bo = np.asarray(inputs["Wo"]), np.asarray(inputs["bo"])

    # xt tiled: [KS, B*NCH, P, TC]; xt[ks, ch, p, t] = x^T[ks*128+p, ch*512+t]
    xT = np.ascontiguousarray(x.reshape(T, DM).T).astype(f16)
    xt = np.ascontiguousarray(
        xT.reshape(KS, P, B * NCH, TC).transpose(0, 2, 1, 3))

    import ml_dtypes
    f8 = ml_dtypes.float8_e4m3
    # xt8 tiled+interleaved: [KD, B*NCH, P, 2, TC]
    x8 = xT.astype(np.float32).astype(f8)  # from fp16-rounded xT? use x2d
    x8 = np.ascontiguousarray(
        np.asarray(x.reshape(T, DM).T, dtype=np.float32).astype(f8)
        .reshape(KD, 2, P, B * NCH, TC).transpose(0, 3, 2, 1, 4))

    in_maps = []
    for c in range(NCORES):
        sl = slice(c * DC, (c + 1) * DC)
        in_maps.append({
            "xt": xt,
            "xt8": x8,
            "wq8": np.ascontiguousarray(Wq[:, sl] * WSCALE).astype(f8),
            "wk8": np.ascontiguousarray(Wk[:, sl] * WSCALE).astype(f8),
            "wv": np.ascontiguousarray(Wv[:, sl]).astype(f16),
            "bq": np.ascontiguousarray(bq[sl] * WSCALE).astype(np.float32),
            "bk": np.ascontiguousarray(bk[sl] * WSCALE).astype(np.float32),
            "bv": np.ascontiguousarray(bv[sl]).astype(np.float32),
            "wo": np.ascontiguousarray(Wo[sl, :]).astype(f16),
        })

    if _NC_CACHE is None:
        _NC_CACHE = _build_nc()
    res = run_bass_kernel_spmd(_NC_CACHE, in_maps, core_ids=list(range(NCORES)))

    # out tiled [T//P, DM//TC, P, TC] -> [T, DM]
    acc = res.results[0]["out"].astype(np.float32)
    for c in range(1, NCORES):
        acc = acc + res.results[c]["out"].astype(np.float32)
    acc = acc.transpose(0, 2, 1, 3).reshape(T, DM)
    acc = acc + bo[None, :].astype(np.float32)
    return acc.reshape(B, L, DM)
